# revision 19
# baseline (speedup 1.0000x reference)
"""Trainium2 Bass kernel for nn_CrossAttention (chunked local self-attn + full cross-attn).

Sharding: 8 cores = 2 batches x 4 query-row-blocks (512 rows each), fully SPMD,
no collectives.  Phase 1 (LN1 -> qkv -> chunked local attn (CHUNK=64) -> W_ao ->
+residual -> LN2 -> q_in) is query-row-independent.  Phase 2: each core
projects K/V from its batch's full x (4096 keys; 4x redundant within a batch)
and attends its 512 queries over all keys, streamed in 1024-key chunks.

Host-side preprocessing (see prepare_in_maps):
- x is transposed and cast to bf16; all weights cast to bf16.
- LN gammas are folded into the following projection weights
  (Wqkv' = diag(ln1_g) @ W_qkv, Wq' = diag(ln2_g) @ W_q) and the LN betas
  become projection biases (ln1_b @ W_qkv, ln2_b @ W_q), so the device LN
  computes only (x - mean) * rstd.  g2/b2 are applied on the idle Pool
  engine solely for the q_in DRAM output (off the critical path).
- End-to-end rel err ~3e-3 (tolerance 2e-2): bf16 weights + attention
  internals; f32 LN stats, PSUM accumulation, softmax denominators.

Engine budget (per core, timeline-sim validated):
- PE: all matmuls/transposes - the roofline engine.
- Act: exps + half of the PSUM->SBUF copies (Copy is in every act table
  set; the LN Sqrt is batched 1/LN so table swaps are rare).
- DVE: LN stats, reciprocals, other half of PSUM->SBUF copies (biases are
  fused into the copies via tensor_scalar/activation-bias).
- Pool (gpsimd): CANNOT touch PSUM; does SBUF-only work (residual adds,
  q_in g/b epilogue, memsets).
- Chunk-0 K/V projection is hoisted into phase 1 (interleaved with local
  attention / W_ao) so PE has filler work while LN2 and copies run; its
  inputs (Wkv, xT chunk 0) are DMA-prefetched right after Wqkv.

Key implementation facts (hardware-validated):
- bf16 matmuls run 1 cycle/row at any moving size (f32r needs >=256).
- HARD RULE: every matmul operand/output must sit at partition base 0.
- Local attention computes scores TRANSPOSED (s^T[k,q] via stationary=kT),
  so exp output feeds AV directly; softmax denominators come free from a
  ones column in the V stationary (row 64 of the [65,*] AV output) and are
  broadcast across partitions with a ones-column matmul.
- PSUM is 8 banks, statically reserved per open pool; same-tag tiles alive
  simultaneously need distinct tags or bufs >= count.
"""

import numpy as np
import ml_dtypes

import concourse.bacc as bacc
import concourse.bass as bass
import concourse.mybir as mybir
import concourse.tile as tile
from concourse.bass_utils import run_bass_kernel_spmd
from concourse.masks import make_identity

F32 = mybir.dt.float32
BF16 = mybir.dt.bfloat16
AF = mybir.ActivationFunctionType
ALU = mybir.AluOpType

H, DH, CHUNK = 8, 64, 64
DIM = 512
INNER = 512
EPS = 1e-5
SCALE = DH ** -0.5

T = 512          # query rows per core
NKT = 4096       # keys (full x length)
NF = DIM // 128  # feature tiles (4)
NT = T // 128    # token tiles per core (4)
NCL = T // CHUNK  # local chunks per core (8)
KT_CHUNK = 1024  # cross-attn key-chunk
N_CHUNKS = NKT // KT_CHUNK
NKTT = KT_CHUNK // 128  # kt tiles per chunk (8)
GSZ = 2
NG = NKTT // GSZ


def _bcast_ap(dram_ap, parts):
    """[N] DRAM vector -> [parts, N] partition-broadcast AP (for DMA)."""
    return bass.AP(
        tensor=dram_ap.tensor,
        offset=dram_ap.offset,
        ap=[[0, parts]] + [list(x) for x in dram_ap.ap],
    )


class _Alt:
    """Round-robin PSUM->SBUF copies between DVE and Act (Pool cannot touch
    PSUM on TRN2; Act Copy is in every act table set -> no table swaps).
    Optional per-partition bias rides free on either engine."""

    def __init__(self, nc):
        self.nc = nc
        self.i = 0

    def copy(self, out, in_, bias=None):
        self.i += 1
        with self.nc.allow_low_precision(reason="bf16 activations"):
            if self.i % 2:
                if bias is None:
                    self.nc.vector.tensor_copy(out, in_)
                else:
                    self.nc.vector.tensor_scalar(out=out, in0=in_, scalar1=bias,
                                                 scalar2=None, op0=ALU.add)
            else:
                if bias is None:
                    self.nc.scalar.copy(out=out, in_=in_)
                else:
                    self.nc.scalar.activation(out=out, in_=in_, func=AF.Identity,
                                              bias=bias)


def _layernorm(nc, pool, x_tiles, eps_tile, prefix, tag_prefix=None):
    """Plain (x-mean)*rstd -> bf16; batched stats: one Sqrt (Act) + one
    reciprocal (DVE) per LN -> act-table swaps are rare."""
    mvv = pool.tile([128, NT, 2], F32, name=f"{prefix}mv", tag="ln_mv", bufs=2)
    for tt in range(NT):
        stats = pool.tile([128, 6], F32, name="ln_stats", tag="ln_stats")
        nc.vector.bn_stats(out=stats, in_=x_tiles[tt])
        nc.vector.bn_aggr(out=mvv[:, tt, :], in_=stats)
    var_ap = mvv[:, :, 1:2]
    nc.scalar.activation(out=var_ap, in_=var_ap, func=AF.Sqrt,
                         bias=eps_tile, scale=1.0)
    nc.vector.reciprocal(var_ap, var_ap)
    out_tiles = []
    for tt in range(NT):
        y = pool.tile([128, DIM], BF16, name=f"{prefix}{tt}",
                      tag=f"{tag_prefix or prefix}{tt}", bufs=1)
        with nc.allow_low_precision(reason="bf16 activations; f32 stats"):
            nc.vector.tensor_scalar(out=y, in0=x_tiles[tt],
                                    scalar1=mvv[:, tt, 0:1],
                                    scalar2=mvv[:, tt, 1:2],
                                    op0=ALU.subtract, op1=ALU.mult)
        out_tiles.append(y)
    return out_tiles


def _transpose_to(nc, ps_pool, alt, ident, src_tiles, dst_tiles):
    """dst[ft][:, tt*128:+128] = src[tt][:, ft*128:+128].T  (PE transposes).
    4 transposes land in one [128,512] PSUM tile -> single copy out."""
    for ft in range(NF):
        tp = ps_pool.tile([128, 4 * 128], src_tiles[0].dtype, name="tposer",
                          tag="tposer")
        for tt in range(len(src_tiles)):
            nc.tensor.transpose(tp[:, tt * 128:(tt + 1) * 128],
                                src_tiles[tt][:, ft * 128:(ft + 1) * 128], ident)
        alt.copy(dst_tiles[ft][:, :], tp)


def build_nc():
    nc = bacc.Bacc(None, target_bir_lowering=False)

    # ---------------- DRAM I/O ----------------
    qx_d = nc.dram_tensor("qx", [T, DIM], F32, kind="ExternalInput")
    xT_d = nc.dram_tensor("xT", [DIM, NKT], BF16, kind="ExternalInput")
    Wqkv_d = nc.dram_tensor("Wqkv", [DIM, 3 * INNER], BF16, kind="ExternalInput")
    qkvb_d = nc.dram_tensor("qkvb", [3 * INNER], F32, kind="ExternalInput")
    Wao_d = nc.dram_tensor("Wao", [INNER, DIM], BF16, kind="ExternalInput")
    Wq_d = nc.dram_tensor("Wq", [DIM, INNER], BF16, kind="ExternalInput")
    qb_d = nc.dram_tensor("qb", [INNER], F32, kind="ExternalInput")
    Wkv_d = nc.dram_tensor("Wkv", [DIM, 2 * INNER], BF16, kind="ExternalInput")
    Wo_d = nc.dram_tensor("Wo", [INNER, DIM], BF16, kind="ExternalInput")
    g2_d = nc.dram_tensor("g2", [DIM], F32, kind="ExternalInput")
    b2_d = nc.dram_tensor("b2", [DIM], F32, kind="ExternalInput")
    bao_d = nc.dram_tensor("bao", [DIM], F32, kind="ExternalInput")
    bo_d = nc.dram_tensor("bo", [DIM], F32, kind="ExternalInput")
    qin_d = nc.dram_tensor("qin", [T, DIM], F32, kind="ExternalOutput")
    outT_d = nc.dram_tensor("outT", [DIM, T], F32, kind="ExternalOutput")

    with tile.TileContext(nc) as tc:
        with tc.tile_pool(name="singles", bufs=1) as singles, \
             tc.tile_pool(name="persist", bufs=1) as persist, \
             tc.tile_pool(name="wx", bufs=1) as wxpool, \
             tc.tile_pool(name="xc", bufs=2) as xc_pool, \
             tc.tile_pool(name="kc", bufs=2) as kc_pool, \
             tc.tile_pool(name="vc", bufs=12) as vc_pool, \
             tc.tile_pool(name="pa", bufs=4) as pa_pool:

            alt = _Alt(nc)

            ident = singles.tile([128, 128], BF16)
            make_identity(nc, ident)
            eps_t = singles.tile([128, 1], F32)
            nc.vector.memset(eps_t, EPS)
            ones_t = singles.tile([1, 64], BF16)
            nc.vector.memset(ones_t, 1.0)

            # biases / epilogue params (gpsimd DMA queue, parallel to sync)
            qkb_col = singles.tile([128, 8], F32)   # q/k bias per partition
            nc.gpsimd.dma_start(out=qkb_col,
                                in_=qkvb_d[0:2 * INNER].rearrange("(m p) -> p m", p=128))
            vb_bc = singles.tile([128, INNER], F32)  # v bias per column
            nc.gpsimd.dma_start(out=vb_bc,
                                in_=_bcast_ap(qkvb_d[2 * INNER:3 * INNER], 128))
            qb_col = singles.tile([128, NF], F32)
            nc.gpsimd.dma_start(out=qb_col,
                                in_=qb_d[:].rearrange("(m p) -> p m", p=128))
            bao_bc = singles.tile([128, DIM], F32)
            nc.gpsimd.dma_start(out=bao_bc, in_=_bcast_ap(bao_d[:], 128))
            g2_bc = singles.tile([128, DIM], F32)
            nc.gpsimd.dma_start(out=g2_bc, in_=_bcast_ap(g2_d[:], 128))
            b2_bc = singles.tile([128, DIM], F32)
            nc.gpsimd.dma_start(out=b2_bc, in_=_bcast_ap(b2_d[:], 128))
            bo_col = singles.tile([128, NF], F32)
            nc.gpsimd.dma_start(out=bo_col, in_=bo_d[:].rearrange("(m p) -> p m", p=128))

            # qcT survives into the cross-attn phase (per-head, base-0)
            qcT = [persist.tile([64, T], BF16, name=f"qcTh{h}", tag=f"qcTh{h}")
                   for h in range(H)]

            # ---- DMA order = consumption order: qx (LN1) -> Wqkv (qkv proj)
            # -> Wkv + x chunk 0 (hoisted K/V proj) -> Wao -> Wq -> Wo.
            qx_t = []
            for tt in range(NT):
                x = xc_pool.tile([128, DIM], F32, name=f"qx{tt}", tag=f"qx{tt}",
                                 bufs=1)
                nc.sync.dma_start(out=x, in_=qx_d[tt * 128:(tt + 1) * 128, :])
                qx_t.append(x)
            Wqkv_sb, Wao_sb, Wq_sb, Wkv_sb, Wo_sb = [], [], [], [], []
            for ft in range(NF):
                w = wxpool.tile([128, 3 * INNER], BF16, name=f"wqkv{ft}", tag=f"wqkv{ft}")
                nc.sync.dma_start(out=w, in_=Wqkv_d[ft * 128:(ft + 1) * 128, :])
                Wqkv_sb.append(w)
            for ft in range(NF):
                w = wxpool.tile([128, 2 * INNER], BF16, name=f"wkv{ft}", tag=f"wkv{ft}")
                nc.sync.dma_start(out=w, in_=Wkv_d[ft * 128:(ft + 1) * 128, :])
                Wkv_sb.append(w)
            xc0 = []
            for ft in range(NF):
                xt = xc_pool.tile([128, KT_CHUNK], BF16, name=f"xTc{ft}", tag=f"xTc{ft}")
                nc.sync.dma_start(out=xt, in_=xT_d[ft * 128:(ft + 1) * 128, 0:KT_CHUNK])
                xc0.append(xt)
            for ft in range(NF):
                w = wxpool.tile([128, DIM], BF16, name=f"wao{ft}", tag=f"wao{ft}")
                nc.sync.dma_start(out=w, in_=Wao_d[ft * 128:(ft + 1) * 128, :])
                Wao_sb.append(w)
            for ft in range(NF):
                w = wxpool.tile([128, INNER], BF16, name=f"wq{ft}", tag=f"wq{ft}")
                nc.sync.dma_start(out=w, in_=Wq_d[ft * 128:(ft + 1) * 128, :])
                Wq_sb.append(w)
            for ft in range(NF):
                w = wxpool.tile([128, DIM], BF16, name=f"wo{ft}", tag=f"wo{ft}")
                nc.sync.dma_start(out=w, in_=Wo_d[ft * 128:(ft + 1) * 128, :])
                Wo_sb.append(w)
            oT_sb = [wxpool.tile([65, T], F32, name=f"oT{h}", tag=f"oT{h}")
                     for h in range(H)]

            # chunk-0 K/V tiles (projected during phase 1)
            kcT0 = [kc_pool.tile([64, KT_CHUNK], BF16, name=f"kcTh{h}",
                                 tag=f"kcTh{h}") for h in range(H)]
            v_aug0 = [vc_pool.tile([128, H, 65], BF16, name="v_aug", tag="v_aug")
                      for _ in range(NKTT)]

            # =================== PHASE 1 ===================
            with tc.tile_pool(name="p1", bufs=1) as p1, \
                 tc.tile_pool(name="p1w", bufs=4) as p1w, \
                 tc.tile_pool(name="psT", bufs=2, space="PSUM") as psT, \
                 tc.tile_pool(name="psMM", bufs=2, space="PSUM") as psMM, \
                 tc.tile_pool(name="psS", bufs=2, space="PSUM") as psS, \
                 tc.tile_pool(name="psAV", bufs=2, space="PSUM") as psAV:

                # ---- A. LN1 (no g/b: folded into Wqkv); B. transpose -> lnT
                ln1 = _layernorm(nc, p1w, qx_t, eps_t, 'ln1_')
                lnT = [p1.tile([128, T], BF16, name=f"lnT{ft}", tag=f"lnT{ft}")
                       for ft in range(NF)]
                _transpose_to(nc, psT, alt, ident, ln1, lnT)

                # ---- C. qkv projections; q/k transposed bf16, v natural bf16
                qT = [p1.tile([64, T], BF16, name=f"qTh{h}", tag=f"qTh{h}") for h in range(H)]
                kT = [p1.tile([64, T], BF16, name=f"kTh{h}", tag=f"kTh{h}") for h in range(H)]
                for m in range(8):  # 4 q tiles + 4 k tiles (transposed outputs)
                    ps = psMM.tile([128, T], F32, name="proj_ps", tag="proj_ps")
                    for ft in range(NF):
                        nc.tensor.matmul(ps[:, :],
                                         Wqkv_sb[ft][:, m * 128:(m + 1) * 128],
                                         lnT[ft][:, :],
                                         start=(ft == 0), stop=(ft == NF - 1))
                    dst = qT if m < 4 else kT
                    mm = m % 4
                    alt.copy(dst[2 * mm], ps[0:64, :], bias=qkb_col[0:64, m:m + 1])
                    alt.copy(dst[2 * mm + 1], ps[64:128, :],
                             bias=qkb_col[64:128, m:m + 1])
                # v with ones column: va_loc[c] [64, H, 65] bf16 (AV stationary)
                va_loc = [p1.tile([64, H, 65], BF16, name=f"va{c}", tag=f"va{c}")
                          for c in range(NCL)]
                for c in range(NCL):
                    nc.gpsimd.memset(va_loc[c][:, :, 64:65], 1.0)
                vb_r = vb_bc[0:64, :].rearrange("p (h d) -> p h d", h=H)
                for tt in range(NT):
                    ps = psMM.tile([128, INNER], F32, name="proj_ps", tag="proj_ps")
                    for ft in range(NF):
                        nc.tensor.matmul(ps[:, :],
                                         lnT[ft][:, tt * 128:(tt + 1) * 128],
                                         Wqkv_sb[ft][:, 2 * INNER:3 * INNER],
                                         start=(ft == 0), stop=(ft == NF - 1))
                    with nc.allow_low_precision(reason="bf16 v"):
                        nc.vector.tensor_tensor(
                            out=va_loc[2 * tt][:, :, 0:64],
                            in0=ps[0:64, :].rearrange("p (h d) -> p h d", h=H),
                            in1=vb_r, op=ALU.add)
                        nc.vector.tensor_tensor(
                            out=va_loc[2 * tt + 1][:, :, 0:64],
                            in0=ps[64:128, :].rearrange("p (h d) -> p h d", h=H),
                            in1=vb_r, op=ALU.add)

                # ---- D. chunked local attention -> oT_local (transposed bf16)
                # s^T[k,q] per 64-chunk: stationary kT slice, moving qT slice.
                # Interleaved with chunk-0 K^T projection as PE filler work.
                oT_local = [p1.tile([128, T], BF16, name=f"oTl{m}", tag=f"oTl{m}")
                            for m in range(NF)]
                for h in range(H):
                    sT_ps = psS.tile([64, NCL, CHUNK], F32, name="sT", tag="sT")
                    for c in range(NCL):
                        sl = slice(c * CHUNK, (c + 1) * CHUNK)
                        nc.tensor.matmul(sT_ps[:, c, :], kT[h][:, sl], qT[h][:, sl],
                                         start=True, stop=True, tile_position=(0, 0))
                    aT = p1w.tile([64, NCL, CHUNK], BF16, name="aT", tag="aT", bufs=2)
                    nc.scalar.activation(out=aT, in_=sT_ps, func=AF.Exp, scale=SCALE)
                    av_ps = psAV.tile([65, NCL, CHUNK], F32, name="avT", tag="avT")
                    for c in range(NCL):
                        nc.tensor.matmul(av_ps[:, c, :], va_loc[c][:, h, :],
                                         aT[:, c, :],
                                         start=True, stop=True, tile_position=(0, 0))
                    rec = p1w.tile([1, NCL, CHUNK], BF16, name="rec_l", tag="rec_l",
                                   bufs=2)
                    with nc.allow_low_precision(reason="bf16 softmax denominators"):
                        nc.vector.reciprocal(rec, av_ps[64:65, :, :])
                    bc_ps = psMM.tile([64, NCL, CHUNK], F32, name="proj_ps",
                                      tag="proj_ps")
                    nc.tensor.matmul(bc_ps[:, :, :], ones_t[0:1, :], rec[0:1, :, :],
                                     start=True, stop=True)
                    hp, hr = h // 2, (h % 2) * 64
                    dst = oT_local[hp][hr:hr + 64, :].rearrange(
                        "p (c k) -> p c k", c=NCL)
                    alt.copy(dst, av_ps[0:64, :, :])
                    with nc.allow_low_precision(reason="bf16 local attn out"):
                        nc.vector.tensor_tensor(out=dst, in0=dst, in1=bc_ps,
                                                op=ALU.mult)
                    # filler: chunk-0 K^T projection slice (no deps on local attn)
                    kc_ps = psMM.tile([128, 512], F32, name="proj_ps", tag="proj_ps")
                    m, nchunk = h // 2, h % 2
                    for ft in range(NF):
                        nc.tensor.matmul(
                            kc_ps[:, :],
                            Wkv_sb[ft][:, m * 128:(m + 1) * 128],
                            xc0[ft][:, nchunk * 512:(nchunk + 1) * 512],
                            start=(ft == 0), stop=(ft == NF - 1))
                    sl = slice(nchunk * 512, (nchunk + 1) * 512)
                    alt.copy(kcT0[2 * m][:, sl], kc_ps[0:64, :])
                    alt.copy(kcT0[2 * m + 1][:, sl], kc_ps[64:128, :])

                # ---- E. W_ao projection (oT_local stationary -> natural out)
                # + bias + residual; interleaved with chunk-0 V projection.
                ao = [p1.tile([128, DIM], F32, name=f"ao{tt}", tag=f"ao{tt}")
                      for tt in range(NT)]
                for tt in range(NT):
                    ps = psMM.tile([128, T], F32, name="proj_ps", tag="proj_ps")
                    for ft in range(NF):
                        nc.tensor.matmul(ps[:, :],
                                         oT_local[ft][:, tt * 128:(tt + 1) * 128],
                                         Wao_sb[ft][:, :],
                                         start=(ft == 0), stop=(ft == NF - 1))
                    nc.vector.tensor_tensor(out=ao[tt], in0=ps, in1=bao_bc, op=ALU.add)
                    nc.gpsimd.tensor_tensor(out=ao[tt], in0=ao[tt], in1=qx_t[tt],
                                            op=ALU.add)
                    # filler: chunk-0 V projection (2 kt tiles per tt)
                    for kt in (2 * tt, 2 * tt + 1):
                        vp = psMM.tile([128, INNER], F32, name="proj_ps", tag="proj_ps")
                        for ft in range(NF):
                            nc.tensor.matmul(
                                vp[:, :],
                                xc0[ft][:, kt * 128:(kt + 1) * 128],
                                Wkv_sb[ft][:, INNER:2 * INNER],
                                start=(ft == 0), stop=(ft == NF - 1))
                        nc.vector.tensor_copy(
                            v_aug0[kt][:, :, 0:64],
                            vp[:, :].rearrange("p (h d) -> p h d", h=H))
                        nc.gpsimd.memset(v_aug0[kt][:, :, 64:65], 1.0)

                # ---- F. LN2 -> qin_raw (bf16, g2/b2 folded into Wq);
                # qin output epilogue (g2*x+b2) on Pool, off the critical path
                qin = _layernorm(nc, p1w, ao, eps_t, 'qin_', tag_prefix='ln1_')
                for tt in range(NT):
                    qo = p1.tile([128, DIM], F32, name=f"qo{tt}", tag=f"qo{tt}")
                    nc.gpsimd.tensor_tensor(out=qo, in0=qin[tt], in1=g2_bc,
                                            op=ALU.mult)
                    nc.gpsimd.tensor_tensor(out=qo, in0=qo, in1=b2_bc, op=ALU.add)
                    nc.sync.dma_start(out=qin_d[tt * 128:(tt + 1) * 128, :], in_=qo)
                qinT = [p1.tile([128, T], BF16, name=f"qinT{ft}", tag=f"lnT{ft}")
                        for ft in range(NF)]
                _transpose_to(nc, psT, alt, ident, qin, qinT)

                # ---- H. W_q projection -> qcT (persists, bf16)
                for m in range(NF):
                    ps = psMM.tile([128, T], F32, name="proj_ps", tag="proj_ps")
                    for ft in range(NF):
                        nc.tensor.matmul(ps[:, :],
                                         Wq_sb[ft][:, m * 128:(m + 1) * 128],
                                         qinT[ft][:, :],
                                         start=(ft == 0), stop=(ft == NF - 1))
                    alt.copy(qcT[2 * m], ps[0:64, :], bias=qb_col[0:64, m:m + 1])
                    alt.copy(qcT[2 * m + 1], ps[64:128, :],
                             bias=qb_col[64:128, m:m + 1])

            # =================== PHASE 2: cross-attention ===================
            with tc.tile_pool(name="ps_s", bufs=2, space="PSUM") as ps_s, \
                 tc.tile_pool(name="ps_o", bufs=2, space="PSUM") as ps_o, \
                 tc.tile_pool(name="ps_p", bufs=2, space="PSUM") as ps_p:
                for chunk in range(N_CHUNKS):
                    k0 = chunk * KT_CHUNK
                    if chunk == 0:
                        kcT, v_aug = kcT0, v_aug0
                    else:
                        xTc = []
                        for ft in range(NF):
                            xt = xc_pool.tile([128, KT_CHUNK], BF16,
                                              name=f"xTc{ft}", tag=f"xTc{ft}")
                            nc.sync.dma_start(
                                out=xt, in_=xT_d[ft * 128:(ft + 1) * 128, k0:k0 + KT_CHUNK])
                            xTc.append(xt)
                        # K^T projection -> kcT[h] [64, kt] bf16
                        kcT = [kc_pool.tile([64, KT_CHUNK], BF16, name=f"kcTh{h}",
                                            tag=f"kcTh{h}") for h in range(H)]
                        for m in range(NF):
                            for nchunk in range(KT_CHUNK // 512):
                                kc_ps = ps_p.tile([128, 512], F32, name="proj_ps",
                                                  tag="proj_ps")
                                for ft in range(NF):
                                    nc.tensor.matmul(
                                        kc_ps[:, :],
                                        Wkv_sb[ft][:, m * 128:(m + 1) * 128],
                                        xTc[ft][:, nchunk * 512:(nchunk + 1) * 512],
                                        start=(ft == 0), stop=(ft == NF - 1))
                                sl = slice(nchunk * 512, (nchunk + 1) * 512)
                                nc.vector.tensor_copy(kcT[2 * m][:, sl], kc_ps[0:64, :])
                                nc.vector.tensor_copy(kcT[2 * m + 1][:, sl],
                                                      kc_ps[64:128, :])
                        # V projection (natural) + ones column -> v_aug bf16
                        v_aug = []
                        for kt in range(NKTT):
                            vp = ps_p.tile([128, INNER], F32, name="proj_ps",
                                           tag="proj_ps")
                            for ft in range(NF):
                                nc.tensor.matmul(
                                    vp[:, :],
                                    xTc[ft][:, kt * 128:(kt + 1) * 128],
                                    Wkv_sb[ft][:, INNER:2 * INNER],
                                    start=(ft == 0), stop=(ft == NF - 1))
                            va = vc_pool.tile([128, H, 65], BF16, name="v_aug",
                                              tag="v_aug")
                            nc.vector.tensor_copy(
                                va[:, :, 0:64],
                                vp[:, :].rearrange("p (h d) -> p h d", h=H))
                            nc.gpsimd.memset(va[:, :, 64:65], 1.0)
                            v_aug.append(va)
                    # attention per head
                    for h in range(H):
                        o_ps = ps_o.tile([65, T], F32, name="o_ps", tag="o_ps")
                        for g in range(NG):  # groups of GSZ kt-tiles
                            s_ps = ps_s.tile([128, GSZ, T], F32, name="s_ps",
                                             tag="s_ps")
                            for j in range(GSZ):
                                kt = g * GSZ + j
                                nc.tensor.matmul(
                                    s_ps[:, j, :],
                                    kcT[h][:, kt * 128:(kt + 1) * 128],
                                    qcT[h][:, :],
                                    start=True, stop=True,
                                    tile_position=(0, 0))
                            a_sb = pa_pool.tile([128, GSZ, T], BF16, name="a_sb",
                                                tag="a_sb")
                            nc.scalar.activation(out=a_sb, in_=s_ps, func=AF.Exp,
                                                 scale=SCALE)
                            for j in range(GSZ):
                                kt = g * GSZ + j
                                nc.tensor.matmul(
                                    o_ps[:, :],
                                    v_aug[kt][:, h, :],
                                    a_sb[:, j, :],
                                    start=(g == 0 and j == 0),
                                    stop=(g == NG - 1 and j == GSZ - 1))
                        if chunk == 0:
                            nc.vector.tensor_copy(oT_sb[h], o_ps)
                        else:
                            nc.vector.tensor_tensor(out=oT_sb[h], in0=oT_sb[h],
                                                    in1=o_ps, op=ALU.add)

            # =================== normalize + W_o ===================
            with tc.tile_pool(name="fin", bufs=1) as fin, \
                 tc.tile_pool(name="ps_f", bufs=2, space="PSUM") as ps_f:
                oT_norm = [fin.tile([128, T], BF16, name=f"oTn{m}", tag=f"oTn{m}")
                           for m in range(NF)]
                for h in range(H):
                    hp, hr = h // 2, (h % 2) * 64
                    rec = fin.tile([1, T], BF16, name="rec", tag="rec", bufs=2)
                    with nc.allow_low_precision(reason="bf16 softmax denominators"):
                        nc.vector.reciprocal(rec, oT_sb[h][64:65, :])
                    bc_ps = ps_f.tile([64, T], F32, name="bc_ps", tag="bc_ps")
                    nc.tensor.matmul(bc_ps[:, :], ones_t[0:1, :], rec[0:1, :],
                                     start=True, stop=True)
                    with nc.allow_low_precision(reason="bf16 attn out"):
                        nc.vector.tensor_tensor(out=oT_norm[hp][hr:hr + 64, :],
                                                in0=oT_sb[h][0:64, :], in1=bc_ps,
                                                op=ALU.mult)
                for m in range(NF):
                    ps = ps_f.tile([128, T], F32, name="out_ps", tag="out_ps")
                    for ft in range(NF):
                        nc.tensor.matmul(ps[:, :],
                                         Wo_sb[ft][:, m * 128:(m + 1) * 128],
                                         oT_norm[ft][:, :],
                                         start=(ft == 0), stop=(ft == NF - 1))
                    ot = fin.tile([128, T], F32, name="outT_sb", tag="outT_sb", bufs=2)
                    nc.vector.tensor_scalar(out=ot, in0=ps, scalar1=bo_col[:, m:m + 1],
                                            scalar2=None, op0=ALU.add)
                    nc.sync.dma_start(out=outT_d[m * 128:(m + 1) * 128, :], in_=ot)

    nc.finalize()
    return nc


_NC_CACHE = {}

BF = ml_dtypes.bfloat16


def prepare_in_maps(x, q_x, ln1_g, ln1_b, W_qkv, W_ao, b_ao, ln2_g, ln2_b,
                    W_q, W_kv, W_o, b_o):
    """Host-side preprocessing: transpose/cast x, fold LN1/LN2 gammas+betas
    into the qkv / q projection weights, cast weights to bf16."""
    x = np.asarray(x, np.float32)
    q_x = np.asarray(q_x, np.float32)
    xT = np.ascontiguousarray(x.transpose(0, 2, 1)).astype(BF)
    Wqkv = np.asarray(W_qkv, np.float32)
    Wq = np.asarray(W_q, np.float32)
    Wqkv_f = np.asarray(ln1_g, np.float32)[:, None] * Wqkv
    qkvb = np.asarray(ln1_b, np.float32) @ Wqkv
    Wq_f = np.asarray(ln2_g, np.float32)[:, None] * Wq
    qb = np.asarray(ln2_b, np.float32) @ Wq
    common = {
        "Wqkv": Wqkv_f.astype(BF),
        "qkvb": qkvb.astype(np.float32),
        "Wao": np.asarray(W_ao).astype(BF),
        "Wq": Wq_f.astype(BF),
        "qb": qb.astype(np.float32),
        "Wkv": np.asarray(W_kv).astype(BF),
        "Wo": np.asarray(W_o).astype(BF),
        "g2": np.asarray(ln2_g, np.float32),
        "b2": np.asarray(ln2_b, np.float32),
        "bao": np.asarray(b_ao, np.float32),
        "bo": np.asarray(b_o, np.float32),
    }
    in_maps = []
    for c in range(8):
        b, r = c // 4, c % 4
        m = dict(common)
        m["qx"] = np.ascontiguousarray(q_x[b, r * T:(r + 1) * T, :], np.float32)
        m["xT"] = xT[b]
        in_maps.append(m)
    return in_maps


def kernel(x, q_x, ln1_g, ln1_b, W_qkv, W_ao, b_ao, ln2_g, ln2_b,
           W_q, W_kv, W_o, b_o):
    B, NQ, _ = q_x.shape

    if "nc" not in _NC_CACHE:
        _NC_CACHE["nc"] = build_nc()
    nc = _NC_CACHE["nc"]

    in_maps = prepare_in_maps(x, q_x, ln1_g, ln1_b, W_qkv, W_ao, b_ao,
                              ln2_g, ln2_b, W_q, W_kv, W_o, b_o)
    res = run_bass_kernel_spmd(nc, in_maps, core_ids=list(range(8)))

    out = np.empty((B, NQ, DIM), np.float32)
    q_in = np.empty((B, NQ, DIM), np.float32)
    for c in range(8):
        b, r = c // 4, c % 4
        q_in[b, r * T:(r + 1) * T, :] = res.results[c]["qin"]
        out[b, r * T:(r + 1) * T, :] = res.results[c]["outT"].T
    return (out, q_in)
